# revision 3
# baseline (speedup 1.0000x reference)
"""Trainium2 Bass kernel for MLP-as-GNN: out = relu(x @ W1.T + b1) @ W2.T + b2.

Shapes (full): x [8, 4096, 1024] f32, W1 [4096, 1024], b1 [4096],
W2 [1024, 4096], b2 [1024] -> out [8, 4096, 1024] f32.

Strategy: data-parallel over the batch dim (8 cores, one batch element each).
Per core: M=4096 tokens, two chained GEMMs with the hidden activation kept
on-chip. Matmuls run in fp16 (1 cycle/row on the PE vs 4 for fp32) with fp32
PSUM accumulation; measured rel err vs the f32 reference ~4e-4.

Layout trick: host pre-transposes so the contraction dim lands on SBUF
partitions with no on-device transposes anywhere:
  GEMM1: lhsT = W1T tile [f,128j], rhs = xT tile [f, MB]  -> psum hT [j, MB]
         ACT: relu(psum + b1[j]) -> SBUF hT (fp16)
  GEMM2: lhsT = W2T tile [j,128o], rhs = hT tile [j, MB]  -> psum outT [o, MB]
         ACT: psum + b2[o] -> SBUF outT (f32) -> DRAM
Host transposes outT [1024, 4096] back to [4096, 1024] per batch element.
"""

import os

import numpy as np

IN_CH, HID, OUT_CH = 1024, 4096, 1024
B, M = 8, 4096
N_CORES = 8
P = 128
MB = 512  # token block (PE moving free dim; 1 full PSUM bank in fp32)

KS1 = IN_CH // P  # 8  k-subtiles in GEMM1
JT = HID // P  # 32 j tiles (GEMM1 output partitions / GEMM2 contraction)
OT = OUT_CH // P  # 8  o tiles
NBLK = M // MB  # 8  token blocks per core

_CACHE = {}
LAST_RESULT = None  # BassKernelResults of the most recent run (for test harness)


def _build_nc():
    import concourse.bass as bass  # noqa: F401
    import concourse.tile as tile
    from concourse import bacc, mybir

    f16 = mybir.dt.float16
    f32 = mybir.dt.float32
    Act = mybir.ActivationFunctionType

    nc = bacc.Bacc("TRN2", target_bir_lowering=False, debug=False)

    xT = nc.dram_tensor("xT", [IN_CH, M], f16, kind="ExternalInput").ap()
    w1T = nc.dram_tensor("w1T", [IN_CH, HID], f16, kind="ExternalInput").ap()
    w2T = nc.dram_tensor("w2T", [HID, OUT_CH], f16, kind="ExternalInput").ap()
    b1d = nc.dram_tensor("b1t", [P, JT], f32, kind="ExternalInput").ap()
    b2d = nc.dram_tensor("b2t", [P, OT], f32, kind="ExternalInput").ap()
    outT = nc.dram_tensor("outT", [OUT_CH, M], f32, kind="ExternalOutput").ap()

    xT3 = xT.rearrange("(ko p) m -> p ko m", p=P)
    w1T3 = w1T.rearrange("(ko p) j -> p ko j", p=P)
    w2T3 = w2T.rearrange("(ks p) o -> p ks o", p=P)
    outT3 = outT.rearrange("(os p) m -> p os m", p=P)

    with tile.TileContext(nc) as tc:
        with (
            tc.tile_pool(name="consts", bufs=1) as consts,
            tc.tile_pool(name="xpool", bufs=2) as xpool,
            tc.tile_pool(name="hpool", bufs=1) as hpool,
            tc.tile_pool(name="opool", bufs=1) as opool,
            tc.tile_pool(name="psum1", bufs=4, space="PSUM") as psum1,
            tc.tile_pool(name="psum2", bufs=4, space="PSUM") as psum2,
        ):
            # Chunked weight loads: 1 MB pieces in consumption order, so the
            # first matmul only waits for its own chunk and the PE never sits
            # idle long enough for HAM to re-throttle the clock.
            w1s = consts.tile([P, KS1, HID], f16, name="w1s", tag="w1s")
            w2s = consts.tile([P, JT, OUT_CH], f16, name="w2s", tag="w2s")
            b1s = consts.tile([P, JT], f32, name="b1s", tag="b1s")
            b2s = consts.tile([P, OT], f32, name="b2s", tag="b2s")
            W1CH, W2CH = 512, 128  # j / o elements per chunk (1 MB each)
            nc.sync.dma_start(w1s[:, :, 0:W1CH], w1T3[:, :, 0:W1CH])
            nc.sync.dma_start(b1s, b1d)
            nc.sync.dma_start(b2s, b2d)
            for c in range(1, HID // W1CH):
                csl = slice(c * W1CH, (c + 1) * W1CH)
                nc.sync.dma_start(w1s[:, :, csl], w1T3[:, :, csl])
            for c in range(OUT_CH // W2CH):
                csl = slice(c * W2CH, (c + 1) * W2CH)
                nc.sync.dma_start(w2s[:, :, csl], w2T3[:, :, csl])

            for mb in range(NBLK):
                msl = slice(mb * MB, (mb + 1) * MB)
                xt = xpool.tile([P, KS1, MB], f16, name="xt", tag="xt")
                nc.sync.dma_start(xt, xT3[:, :, msl])

                ht = hpool.tile([P, JT, MB], f16, name="ht", tag="ht")
                for jt in range(JT):
                    ps = psum1.tile([P, MB], mybir.dt.float32, name="ps1", tag="ps1")
                    for k in range(KS1):
                        nc.tensor.matmul(
                            ps,
                            w1s[:, k, jt * P : (jt + 1) * P],
                            xt[:, k, :],
                            start=(k == 0),
                            stop=(k == KS1 - 1),
                        )
                    nc.scalar.activation(
                        ht[:, jt, :], ps, Act.Relu, bias=b1s[:, jt : jt + 1]
                    )

                ot = opool.tile([P, OT, MB], f32, name="ot", tag="ot")
                for o in range(OT):
                    ps2 = psum2.tile([P, MB], mybir.dt.float32, name="ps2", tag="ps2")
                    for ks in range(JT):
                        nc.tensor.matmul(
                            ps2,
                            w2s[:, ks, o * P : (o + 1) * P],
                            ht[:, ks, :],
                            start=(ks == 0),
                            stop=(ks == JT - 1),
                        )
                    nc.scalar.activation(
                        ot[:, o, :], ps2, Act.Identity, bias=b2s[:, o : o + 1]
                    )
                nc.sync.dma_start(outT3[:, :, msl], ot)

    nc.compile()
    return nc


def kernel(x, W1, b1, W2, b2):
    global LAST_RESULT
    from concourse.bass_utils import run_bass_kernel_spmd

    if "nc" not in _CACHE:
        _CACHE["nc"] = _build_nc()
    nc = _CACHE["nc"]

    w1T = np.ascontiguousarray(W1.astype(np.float16).T)  # [1024 f, 4096 j]
    w2T = np.ascontiguousarray(W2.astype(np.float16).T)  # [4096 j, 1024 o]
    b1t = np.ascontiguousarray(b1.astype(np.float32).reshape(JT, P).T)  # [p, jt]
    b2t = np.ascontiguousarray(b2.astype(np.float32).reshape(OT, P).T)  # [p, ot]

    in_maps = []
    for c in range(N_CORES):
        xTc = np.ascontiguousarray(x[c].astype(np.float16).T)  # [1024 f, 4096 m]
        in_maps.append({"xT": xTc, "w1T": w1T, "w2T": w2T, "b1t": b1t, "b2t": b2t})

    LAST_RESULT = run_bass_kernel_spmd(
        nc,
        in_maps,
        core_ids=list(range(N_CORES)),
        trace=bool(int(os.environ.get("KERNEL_TRACE", "0"))),
    )

    out = np.empty((B, M, OUT_CH), dtype=np.float32)
    for c in range(N_CORES):
        out[c] = LAST_RESULT.results[c]["outT"].T
    return out


# revision 4
# speedup vs baseline: 1.0659x; 1.0659x over previous
"""Trainium2 Bass kernel for MLP-as-GNN: out = relu(x @ W1.T + b1) @ W2.T + b2.

Shapes (full): x [8, 4096, 1024] f32, W1 [4096, 1024], b1 [4096],
W2 [1024, 4096], b2 [1024] -> out [8, 4096, 1024] f32.

Strategy: data-parallel over the batch dim (8 cores, one batch element each).
Per core: M=4096 tokens, two chained GEMMs with the hidden activation kept
on-chip. Matmuls run in fp16 (1 cycle/row on the PE vs 4 for fp32) with fp32
PSUM accumulation; measured rel err vs the f32 reference ~4e-4.

Layout trick: host pre-transposes so the contraction dim lands on SBUF
partitions with no on-device transposes anywhere:
  GEMM1: lhsT = W1T tile [f,128j], rhs = xT tile [f, MB]  -> psum hT [j, MB]
         ACT: relu(psum + b1[j]) -> SBUF hT (fp16)
  GEMM2: lhsT = W2T tile [j,128o], rhs = hT tile [j, MB]  -> psum outT [o, MB]
         ACT: psum + b2[o] -> SBUF outT (f32) -> DRAM
Host transposes outT [1024, 4096] back to [4096, 1024] per batch element.
"""

import os

import numpy as np

IN_CH, HID, OUT_CH = 1024, 4096, 1024
B, M = 8, 4096
N_CORES = 8
P = 128
MB = 512  # token block (PE moving free dim; 1 full PSUM bank in fp32)

KS1 = IN_CH // P  # 8  k-subtiles in GEMM1
JT = HID // P  # 32 j tiles (GEMM1 output partitions / GEMM2 contraction)
OT = OUT_CH // P  # 8  o tiles
NBLK = M // MB  # 8  token blocks per core

_CACHE = {}
LAST_RESULT = None  # BassKernelResults of the most recent run (for test harness)


def _build_nc():
    import concourse.bass as bass  # noqa: F401
    import concourse.tile as tile
    from concourse import bacc, mybir

    f16 = mybir.dt.float16
    f32 = mybir.dt.float32
    Act = mybir.ActivationFunctionType

    nc = bacc.Bacc("TRN2", target_bir_lowering=False, debug=False)

    xT = nc.dram_tensor("xT", [IN_CH, M], f16, kind="ExternalInput").ap()
    w1T = nc.dram_tensor("w1T", [IN_CH, HID], f16, kind="ExternalInput").ap()
    w2T = nc.dram_tensor("w2T", [HID, OUT_CH], f16, kind="ExternalInput").ap()
    b1d = nc.dram_tensor("b1t", [P, JT], f32, kind="ExternalInput").ap()
    b2d = nc.dram_tensor("b2t", [P, OT], f32, kind="ExternalInput").ap()
    outT = nc.dram_tensor("outT", [OUT_CH, M], f32, kind="ExternalOutput").ap()

    xT3 = xT.rearrange("(ko p) m -> p ko m", p=P)
    w1T3 = w1T.rearrange("(ko p) j -> p ko j", p=P)
    w2T3 = w2T.rearrange("(ks p) o -> p ks o", p=P)
    outT3 = outT.rearrange("(os p) m -> p os m", p=P)

    with tile.TileContext(nc) as tc:
        with (
            tc.tile_pool(name="consts", bufs=1) as consts,
            tc.tile_pool(name="xpool", bufs=2) as xpool,
            tc.tile_pool(name="hpool", bufs=1) as hpool,
            tc.tile_pool(name="opool", bufs=1) as opool,
            tc.tile_pool(name="psum1", bufs=4, space="PSUM") as psum1,
            tc.tile_pool(name="psum2", bufs=4, space="PSUM") as psum2,
        ):
            # Chunked weight loads: 1 MB pieces in consumption order, so the
            # first matmul only waits for its own chunk and the PE never sits
            # idle long enough for HAM to re-throttle the clock.
            w1s = consts.tile([P, KS1, HID], f16, name="w1s", tag="w1s")
            w2s = consts.tile([P, JT, OUT_CH], f16, name="w2s", tag="w2s")
            b1s = consts.tile([P, JT], f32, name="b1s", tag="b1s")
            b2s = consts.tile([P, OT], f32, name="b2s", tag="b2s")
            # DMA triggers serialize on the Sync engine at ~350 GB/s, so
            # issue order = arrival order. xt(0) + w1 chunk 0 first (the
            # first matmul's only inputs), then the rest in consumption
            # order so compute streams ahead of the loads.
            W1CH, W2CH = 512, 128  # j / o elements per chunk (1 MB each)
            xt0 = xpool.tile([P, KS1, MB], f16, name="xt", tag="xt")
            nc.sync.dma_start(xt0, xT3[:, :, 0:MB])
            nc.sync.dma_start(w1s[:, :, 0:W1CH], w1T3[:, :, 0:W1CH])
            nc.sync.dma_start(b1s, b1d)
            nc.sync.dma_start(b2s, b2d)
            for c in range(1, HID // W1CH):
                csl = slice(c * W1CH, (c + 1) * W1CH)
                nc.sync.dma_start(w1s[:, :, csl], w1T3[:, :, csl])
            for c in range(OUT_CH // W2CH):
                csl = slice(c * W2CH, (c + 1) * W2CH)
                nc.sync.dma_start(w2s[:, :, csl], w2T3[:, :, csl])

            for mb in range(NBLK):
                msl = slice(mb * MB, (mb + 1) * MB)
                if mb == 0:
                    xt = xt0
                else:
                    xt = xpool.tile([P, KS1, MB], f16, name="xt", tag="xt")
                    nc.sync.dma_start(xt, xT3[:, :, msl])

                ht = hpool.tile([P, JT, MB], f16, name="ht", tag="ht")
                for jt in range(JT):
                    ps = psum1.tile([P, MB], mybir.dt.float32, name="ps1", tag="ps1")
                    for k in range(KS1):
                        nc.tensor.matmul(
                            ps,
                            w1s[:, k, jt * P : (jt + 1) * P],
                            xt[:, k, :],
                            start=(k == 0),
                            stop=(k == KS1 - 1),
                        )
                    nc.scalar.activation(
                        ht[:, jt, :], ps, Act.Relu, bias=b1s[:, jt : jt + 1]
                    )

                ot = opool.tile([P, OT, MB], f32, name="ot", tag="ot")
                for o in range(OT):
                    ps2 = psum2.tile([P, MB], mybir.dt.float32, name="ps2", tag="ps2")
                    for ks in range(JT):
                        nc.tensor.matmul(
                            ps2,
                            w2s[:, ks, o * P : (o + 1) * P],
                            ht[:, ks, :],
                            start=(ks == 0),
                            stop=(ks == JT - 1),
                        )
                    nc.scalar.activation(
                        ot[:, o, :], ps2, Act.Identity, bias=b2s[:, o : o + 1]
                    )
                nc.sync.dma_start(outT3[:, :, msl], ot)

    nc.compile()
    return nc


def kernel(x, W1, b1, W2, b2):
    global LAST_RESULT
    from concourse.bass_utils import run_bass_kernel_spmd

    if "nc" not in _CACHE:
        _CACHE["nc"] = _build_nc()
    nc = _CACHE["nc"]

    w1T = np.ascontiguousarray(W1.astype(np.float16).T)  # [1024 f, 4096 j]
    w2T = np.ascontiguousarray(W2.astype(np.float16).T)  # [4096 j, 1024 o]
    b1t = np.ascontiguousarray(b1.astype(np.float32).reshape(JT, P).T)  # [p, jt]
    b2t = np.ascontiguousarray(b2.astype(np.float32).reshape(OT, P).T)  # [p, ot]

    in_maps = []
    for c in range(N_CORES):
        xTc = np.ascontiguousarray(x[c].astype(np.float16).T)  # [1024 f, 4096 m]
        in_maps.append({"xT": xTc, "w1T": w1T, "w2T": w2T, "b1t": b1t, "b2t": b2t})

    LAST_RESULT = run_bass_kernel_spmd(
        nc,
        in_maps,
        core_ids=list(range(N_CORES)),
        trace=bool(int(os.environ.get("KERNEL_TRACE", "0"))),
    )

    out = np.empty((B, M, OUT_CH), dtype=np.float32)
    for c in range(N_CORES):
        out[c] = LAST_RESULT.results[c]["outT"].T
    return out
